# revision 16
# baseline (speedup 1.0000x reference)
"""Distributed Trainium2 Bass kernel for the AttentionBlock problem.

Math (per batch b):
  q = query @ W_Q + b_Q           [B,SQ,H,HS]
  k = key   @ W_K + b_K           [B,SK,H,HS]
  v = value @ W_V + b_V           [B,SK,H,HS]
  scores = (q.k^T)/sqrt(HS) + pos[b,k,h] ; masked keys -> -inf ; dummy col 0
  pattern = softmax(scores)
  z = pattern @ v ; attn = z @ W_O ; out = LayerNorm(attn)

Device-side simplifications (exact math, not approximations):
  * LayerNorm is invariant to a per-row scale of its input, so the softmax
    denominator (and the dummy column, whose value row is 0) cancels:
    LN((P@v)/d @ W_O) == LN(P@v @ W_O).  We never normalize the softmax.
  * Scores are bounded (|scores| < ~10 for this distribution), so exp()
    without max-subtraction is safe in fp32.
  * Masking folds into the exp bias: exp(s/8 + pos + (-1e30 if masked)).

Sharding (8 cores):
  * The 4096 flattened (b, seq) rows are split 512/core; each core computes
    q/k/v projections for its own rows (all heads).
  * k^T and v shards are AllGather'd within each 4-core batch group
    (replica_groups=[[0..3],[4..7]]), so every core sees its full batch at
    identical static offsets (SPMD-safe).
  * Each core then runs attention + out-proj + LayerNorm for its 512 query
    rows.  Output shards concatenate on the host; no all-reduce needed.
"""

import os
from contextlib import ExitStack

import numpy as np

import concourse.bass as bass
import concourse.tile as tile
from concourse import bacc, mybir
from concourse.bass_utils import run_bass_kernel_spmd

B, SQ, SK = 2, 2048, 2048
D = 1024  # QD == KD
H, HS = 16, 64
ED = 1024
NCORES = 8
RPC = B * SQ // NCORES  # 512 rows per core
NKT = SK // 128  # 16 kpos tiles per batch
NDT = D // 128  # 8 contraction tiles
NOT = (H * HS) // 128  # 8 output hs-tiles
NRT = RPC // 128  # 4 row tiles per core
GROUP = NCORES // B  # 4 cores per batch

F32 = mybir.dt.float32
F32R = mybir.dt.float32r
AF = mybir.ActivationFunctionType
ALU = mybir.AluOpType

NEG_BIG = -1.0e30
LN_EPS = 1e-5

LAST_EXEC_NS = None

_CACHED = {}


def _build():
    nc = bacc.Bacc(None, target_bir_lowering=False)

    # Per-core external inputs (host pre-transposed / pre-sliced).
    xqt = nc.dram_tensor("xqt", [D, RPC], F32R, kind="ExternalInput")
    xkt = nc.dram_tensor("xkt", [D, RPC], F32R, kind="ExternalInput")
    xvt = nc.dram_tensor("xvt", [D, RPC], F32R, kind="ExternalInput")
    wq = nc.dram_tensor("wq", [D, H * HS], F32R, kind="ExternalInput")
    wk = nc.dram_tensor("wk", [D, H * HS], F32R, kind="ExternalInput")
    wv = nc.dram_tensor("wv", [D, H * HS], F32R, kind="ExternalInput")
    wo = nc.dram_tensor("wo", [H * HS, ED], F32R, kind="ExternalInput")
    bq = nc.dram_tensor("bq", [128, NOT], F32, kind="ExternalInput")
    bk = nc.dram_tensor("bk", [128, NOT], F32, kind="ExternalInput")
    bv = nc.dram_tensor("bv", [1, H * HS], F32, kind="ExternalInput")
    pos = nc.dram_tensor("pos", [SK, H], F32, kind="ExternalInput")
    maskf = nc.dram_tensor("maskf", [SK], F32, kind="ExternalInput")
    lng = nc.dram_tensor("lng", [1, ED], F32, kind="ExternalInput")
    lnb = nc.dram_tensor("lnb", [1, ED], F32, kind="ExternalInput")
    out = nc.dram_tensor("out", [RPC, ED], F32, kind="ExternalOutput")

    groups = [list(range(GROUP)), list(range(GROUP, 2 * GROUP))]

    with tile.TileContext(nc) as tc, ExitStack() as ctx:
        consts = ctx.enter_context(tc.tile_pool(name="consts", bufs=1))
        xpool = ctx.enter_context(tc.tile_pool(name="xpool", bufs=1))
        wsm = ctx.enter_context(tc.tile_pool(name="wsm", bufs=4))
        wbg = ctx.enter_context(tc.tile_pool(name="wbg", bufs=4))
        evac = ctx.enter_context(tc.tile_pool(name="evac", bufs=4))
        qzpool = ctx.enter_context(tc.tile_pool(name="qzpool", bufs=1))
        ppool = ctx.enter_context(tc.tile_pool(name="ppool", bufs=6))
        kvtiles = ctx.enter_context(tc.tile_pool(name="kvtiles", bufs=6))
        ypool = ctx.enter_context(tc.tile_pool(name="ypool", bufs=4))
        pss = ctx.enter_context(tc.tile_pool(name="pss", bufs=5, space="PSUM"))
        psz = ctx.enter_context(tc.tile_pool(name="psz", bufs=3, space="PSUM"))
        dram = ctx.enter_context(tc.tile_pool(name="dram", bufs=1, space="DRAM"))

        # ---- constants into SBUF ----
        bq_sb = consts.tile([128, NOT], F32)
        nc.sync.dma_start(out=bq_sb, in_=bq[:, :])
        bk_sb = consts.tile([128, NOT], F32)
        nc.sync.dma_start(out=bk_sb, in_=bk[:, :])
        bv_bc = consts.tile([128, H * HS], F32)
        nc.sync.dma_start(out=bv_bc, in_=bv[:, :].to_broadcast([128, H * HS]))
        g_bc = consts.tile([128, ED], F32)
        nc.sync.dma_start(out=g_bc, in_=lng[:, :].to_broadcast([128, ED]))
        b_bc = consts.tile([128, ED], F32)
        nc.sync.dma_start(out=b_bc, in_=lnb[:, :].to_broadcast([128, ED]))
        pos_sb = consts.tile([128, NKT, H], F32)
        nc.sync.dma_start(out=pos_sb, in_=pos[:, :].rearrange("(kt p) h -> p kt h", p=128))
        mask_sb = consts.tile([128, NKT], F32)
        nc.sync.dma_start(out=mask_sb, in_=maskf[:].rearrange("(kt p) -> p kt", p=128))
        eps_sb = consts.tile([128, 1], F32)
        nc.vector.memset(eps_sb, LN_EPS)

        # additive exp-bias per (kpos, head): pos + (mask-1)*1e30
        maskadd = consts.tile([128, NKT], F32)
        nc.vector.tensor_scalar(
            out=maskadd, in0=mask_sb, scalar1=1.0, scalar2=-NEG_BIG,
            op0=ALU.subtract, op1=ALU.mult,
        )
        bias_sb = consts.tile([128, NKT, H], F32)
        for h in range(H):
            nc.vector.tensor_add(
                out=bias_sb[:, :, h], in0=pos_sb[:, :, h], in1=maskadd[:, :]
            )

        # ---- resident activations ----
        xkt_sb = xpool.tile([128, NDT, RPC], F32R)
        nc.sync.dma_start(out=xkt_sb, in_=xkt[:, :].rearrange("(t p) r -> p t r", p=128))
        xvt_sb = xpool.tile([128, NDT, RPC], F32R)
        nc.sync.dma_start(out=xvt_sb, in_=xvt[:, :].rearrange("(t p) r -> p t r", p=128))
        xqt_sb = xpool.tile([128, NDT, RPC], F32R)
        nc.sync.dma_start(out=xqt_sb, in_=xqt[:, :].rearrange("(t p) r -> p t r", p=128))

        qT_sb = qzpool.tile([128, NOT, RPC], F32R)  # q^T  [hs, rows]
        zT_sb = qzpool.tile([128, NOT, RPC], F32R)  # z^T  [hs, rows]

        # DRAM bounce + AllGather outputs
        kT_bounce = dram.tile([H * HS, RPC], F32R)
        v_bounce = dram.tile([RPC, H * 65], F32R)
        kT_full = dram.tile([GROUP * H * HS, RPC], F32R)
        v_full = dram.tile([GROUP * RPC, H * 65], F32R)

        # ---- K projection (transposed layout out) ----
        for t in range(NOT):
            ps = pss.tile([128, RPC], F32, tag="ps")
            for dt in range(NDT):
                wt = wsm.tile([128, 128], F32R, tag="w")
                nc.sync.dma_start(
                    out=wt, in_=wk[dt * 128:(dt + 1) * 128, t * 128:(t + 1) * 128]
                )
                nc.tensor.matmul(
                    ps, lhsT=wt, rhs=xkt_sb[:, dt, :],
                    start=(dt == 0), stop=(dt == NDT - 1),
                )
            kt_ev = evac.tile([128, RPC], F32R, tag="ev")
            nc.vector.tensor_scalar_add(out=kt_ev, in0=ps, scalar1=bk_sb[:, t:t + 1])
            nc.sync.dma_start(out=kT_bounce[t * 128:(t + 1) * 128, :], in_=kt_ev)

        # ---- V projection (natural layout out) ----
        for half in range(2):
            psv = [pss.tile([128, 512], F32, tag="ps", name=f"psv{_i}") for _i in range(NRT)]
            for dt in range(NDT):
                wvt = wbg.tile([128, 512], F32R, tag="w")
                nc.sync.dma_start(
                    out=wvt, in_=wv[dt * 128:(dt + 1) * 128, half * 512:(half + 1) * 512]
                )
                for rt in range(NRT):
                    nc.tensor.matmul(
                        psv[rt],
                        lhsT=xvt_sb[:, dt, rt * 128:(rt + 1) * 128],
                        rhs=wvt,
                        start=(dt == 0), stop=(dt == NDT - 1),
                    )
            for rt in range(NRT):
                v_ev = evac.tile([128, 8, 65], F32R, tag="ev")
                nc.vector.tensor_add(
                    out=v_ev[:, :, 0:64],
                    in0=psv[rt][:, :].rearrange("p (h c) -> p h c", c=64),
                    in1=bv_bc[:, half * 512:(half + 1) * 512].rearrange(
                        "p (h c) -> p h c", c=64),
                )
                nc.vector.memset(v_ev[:, :, 64:65].bitcast(mybir.dt.uint32), 0x3F800000)
                nc.sync.dma_start(
                    out=v_bounce[rt * 128:(rt + 1) * 128,
                                 half * 520:(half + 1) * 520],
                    in_=v_ev,
                )

        # ---- AllGather k^T and v within each batch group ----
        nc.gpsimd.collective_compute(
            "AllGather", ALU.bypass, replica_groups=groups,
            ins=[kT_bounce.opt()], outs=[kT_full.opt()],
        )
        nc.gpsimd.collective_compute(
            "AllGather", ALU.bypass, replica_groups=groups,
            ins=[v_bounce.opt()], outs=[v_full.opt()],
        )

        # ---- Q projection (transposed layout out; overlaps the AllGather) ----
        for t in range(NOT):
            ps = pss.tile([128, RPC], F32, tag="ps")
            for dt in range(NDT):
                wt = wsm.tile([128, 128], F32R, tag="w")
                nc.sync.dma_start(
                    out=wt, in_=wq[dt * 128:(dt + 1) * 128, t * 128:(t + 1) * 128]
                )
                nc.tensor.matmul(
                    ps, lhsT=wt, rhs=xqt_sb[:, dt, :],
                    start=(dt == 0), stop=(dt == NDT - 1),
                )
            nc.vector.tensor_scalar_add(
                out=qT_sb[:, t, :], in0=ps, scalar1=bq_sb[:, t:t + 1]
            )

        # ---- attention, per head-pair (heads 2j at partitions 0:64, 2j+1 at
        # 64:128 -- matching base partitions for lhsT/rhs, and enabling the
        # PE to overlap the two 64-row matmuls in disjoint row groups) ----
        for j in range(H // 2):
            pzs = [psz.tile([65, RPC], F32, tag="pz", name=f"pz{j}_{_i}")
                   for _i in range(2)]
            for kt in range(NKT):
                kpair = kvtiles.tile([128, 128], F32R, tag="kt")
                nc.sync.dma_start(
                    out=kpair,
                    in_=kT_full[
                        1024 * (kt // 4) + 128 * j: 1024 * (kt // 4) + 128 * j + 128,
                        (kt % 4) * 128:(kt % 4) * 128 + 128,
                    ],
                )
                vpair = kvtiles.tile([128, 130], F32R, tag="vt")
                nc.sync.dma_start(
                    out=vpair,
                    in_=v_full[128 * kt:128 * (kt + 1), 130 * j:130 * j + 130],
                )
                for hh in range(2):
                    h, sh = 2 * j + hh, 64 * hh
                    ps_s = pss.tile([128, RPC], F32, tag="ps", name=f"pss{j}_{kt}_{hh}")
                    nc.tensor.matmul(
                        ps_s,
                        lhsT=kpair[sh:sh + 64, :],
                        rhs=qT_sb[sh:sh + 64, j, :],
                        start=True, stop=True,
                    )
                    pt = ppool.tile([128, RPC], F32R, tag="p")
                    nc.scalar.activation(
                        out=pt, in_=ps_s, func=AF.Exp,
                        bias=bias_sb[:, kt, h:h + 1], scale=0.125,
                    )
                    nc.tensor.matmul(
                        pzs[hh],
                        lhsT=vpair[:, 65 * hh:65 * hh + 65],
                        rhs=pt,
                        start=(kt == 0), stop=(kt == NKT - 1),
                        skip_group_check=True,
                    )
            # normalize: d = pz[64] + 1 (dummy col); z /= d.  DVE applies the
            # reciprocal broadcast (psum -> sbuf at base partition 0); a
            # SBUF->SBUF DMA then shifts the odd head to partitions 64:128.
            for hh in range(2):
                d_sb = evac.tile([1, RPC], F32, tag="d", name=f"d{j}_{hh}")
                nc.vector.tensor_scalar_add(
                    out=d_sb, in0=pzs[hh][64:65, :], scalar1=1.0
                )
                r_sb = evac.tile([1, RPC], F32, tag="r", name=f"r{j}_{hh}")
                nc.vector.reciprocal(out=r_sb, in_=d_sb)
                rb_sb = evac.tile([64, RPC], F32, tag="rb", name=f"rb{j}_{hh}")
                nc.gpsimd.partition_broadcast(rb_sb, r_sb)
                if hh == 0:
                    nc.vector.tensor_mul(
                        out=zT_sb[0:64, j, :], in0=pzs[hh][0:64, :], in1=rb_sb
                    )
                else:
                    zn = evac.tile([64, RPC], F32R, tag="zn", name=f"zn{j}")
                    nc.vector.tensor_mul(
                        out=zn, in0=pzs[hh][0:64, :], in1=rb_sb
                    )
                    nc.sync.dma_start(out=zT_sb[64:128, j, :], in_=zn)

        # ---- out projection ----
        y_sb = [ypool.tile([128, ED], F32, tag="y", name=f"ysb{_i}") for _i in range(NRT)]
        for half in range(2):
            psy = [pss.tile([128, 512], F32, tag="ps", name=f"psy{_i}") for _i in range(NRT)]
            for jj in range(NOT):
                wot = wbg.tile([128, 512], F32R, tag="w")
                nc.sync.dma_start(
                    out=wot,
                    in_=wo[jj * 128:(jj + 1) * 128, half * 512:(half + 1) * 512],
                )
                for rt in range(NRT):
                    nc.tensor.matmul(
                        psy[rt],
                        lhsT=zT_sb[:, jj, rt * 128:(rt + 1) * 128],
                        rhs=wot,
                        start=(jj == 0), stop=(jj == NOT - 1),
                    )
            for rt in range(NRT):
                nc.vector.tensor_copy(
                    out=y_sb[rt][:, half * 512:(half + 1) * 512], in_=psy[rt]
                )

        # ---- LayerNorm + store ----
        for rt in range(NRT):
            y = y_sb[rt]
            stats = evac.tile([128, 2, 6], F32, tag="st")
            nc.vector.bn_stats(out=stats[:, 0, :], in_=y[:, 0:512])
            nc.vector.bn_stats(out=stats[:, 1, :], in_=y[:, 512:1024])
            mv = evac.tile([128, 2], F32, tag="mv")
            nc.vector.bn_aggr(out=mv, in_=stats)
            std = evac.tile([128, 1], F32, tag="sd")
            nc.scalar.activation(
                out=std, in_=mv[:, 1:2], func=AF.Sqrt, bias=eps_sb[:, 0:1]
            )
            rstd = evac.tile([128, 1], F32, tag="rs")
            nc.vector.reciprocal(out=rstd, in_=std)
            nc.vector.tensor_scalar(
                out=y, in0=y, scalar1=mv[:, 0:1], scalar2=rstd,
                op0=ALU.subtract, op1=ALU.mult,
            )
            nc.vector.tensor_mul(out=y, in0=y, in1=g_bc)
            nc.vector.tensor_add(out=y, in0=y, in1=b_bc)
            nc.sync.dma_start(out=out[rt * 128:(rt + 1) * 128, :], in_=y)

    return nc


def prep_in_maps(query, key, value, attention_mask, pos_attn_score,
                 W_Q, b_Q, W_K, b_K, W_V, b_V, W_O, ln_gamma, ln_beta):
    f32 = np.float32
    q2 = np.asarray(query, f32).reshape(B * SQ, D)
    k2 = np.asarray(key, f32).reshape(B * SK, D)
    v2 = np.asarray(value, f32).reshape(B * SK, D)
    wq2 = np.ascontiguousarray(np.asarray(W_Q, f32).transpose(2, 1, 0).reshape(D, H * HS))
    wk2 = np.ascontiguousarray(np.asarray(W_K, f32).transpose(2, 1, 0).reshape(D, H * HS))
    wv2 = np.ascontiguousarray(np.asarray(W_V, f32).transpose(2, 1, 0).reshape(D, H * HS))
    wo2 = np.ascontiguousarray(np.asarray(W_O, f32).transpose(1, 2, 0).reshape(H * HS, ED))
    bq2 = np.ascontiguousarray(np.asarray(b_Q, f32).reshape(NOT, 128).T)
    bk2 = np.ascontiguousarray(np.asarray(b_K, f32).reshape(NOT, 128).T)
    bv2 = np.ascontiguousarray(np.asarray(b_V, f32).reshape(1, H * HS))
    pos_np = np.asarray(pos_attn_score, f32)
    mask_np = np.asarray(attention_mask).astype(f32)
    lng = np.ascontiguousarray(np.asarray(ln_gamma, f32).reshape(1, ED))
    lnb = np.ascontiguousarray(np.asarray(ln_beta, f32).reshape(1, ED))

    in_maps = []
    for c in range(NCORES):
        b = c // GROUP
        rows = slice(RPC * c, RPC * (c + 1))
        in_maps.append({
            "xqt": np.ascontiguousarray(q2[rows].T),
            "xkt": np.ascontiguousarray(k2[rows].T),
            "xvt": np.ascontiguousarray(v2[rows].T),
            "wq": wq2, "wk": wk2, "wv": wv2, "wo": wo2,
            "bq": bq2, "bk": bk2, "bv": bv2,
            "pos": np.ascontiguousarray(pos_np[b]),
            "maskf": np.ascontiguousarray(mask_np[b]),
            "lng": lng, "lnb": lnb,
        })
    return in_maps


def kernel(**inputs):
    global LAST_EXEC_NS
    in_maps = prep_in_maps(**inputs)
    if "nc" not in _CACHED:
        nc = _build()
        nc.finalize()
        _CACHED["nc"] = nc
    nc = _CACHED["nc"]

    trace = bool(os.environ.get("BASS_TRACE"))
    res = run_bass_kernel_spmd(nc, in_maps, core_ids=list(range(NCORES)),
                               trace=trace)
    LAST_EXEC_NS = res.exec_time_ns
    _CACHED["last_result"] = res

    out = np.empty((B * SQ, ED), np.float32)
    for c in range(NCORES):
        out[RPC * c:RPC * (c + 1)] = res.results[c]["out"]
    return out.reshape(B, SQ, ED)


# revision 18
# speedup vs baseline: 1.3667x; 1.3667x over previous
"""Distributed Trainium2 Bass kernel for the AttentionBlock problem.

Math (per batch b):
  q/k/v = x @ W + b ; scores = (q.k^T)/8 + pos[b,k,h], masked -> -inf,
  dummy col 0 ; pattern = softmax ; out = LayerNorm((pattern @ v) @ W_O)

Device-side structure:
  * 8 cores, no collectives: the 4096 (b, seq) rows are split 512/core for
    the q path; each core redundantly computes its batch's FULL k/v
    projections (2 GFLOP of bf16 matmul beats a 200+us AllGather).
  * Attention per head-pair (even head on partitions 0:64, odd on 64:128).
  * Softmax: scores are bounded (max ~6), so exp() without max-subtraction
    is safe; mask+pos fold into the exp bias; the dummy column contributes
    exp(0)=1 to the denominator only.  A ones-column appended to each
    65-wide v head block makes the z-matmul accumulate the denominator row
    for free; DVE divides during psum evacuation.
  * All matmul operands are bf16 (PSUM accumulates fp32); softmax/LN
    arithmetic stays fp32.  Verified rel_l2 ~4e-3 vs the fp32 reference.
"""

import os
from contextlib import ExitStack

import numpy as np

import concourse.bass as bass
import concourse.tile as tile
from concourse import bacc, mybir
from concourse.bass_utils import run_bass_kernel_spmd

B, SQ, SK = 2, 2048, 2048
D = 1024  # QD == KD
H, HS = 16, 64
ED = 1024
NCORES = 8
RPC = B * SQ // NCORES  # 512 query rows per core
NKT = SK // 128  # 16 kpos tiles per batch
NDT = D // 128  # 8 contraction tiles
NOT = (H * HS) // 128  # 8 hs-tiles
NRT = RPC // 128  # 4 row tiles per core
NRB = SK // 512  # 4 row blocks per batch
GROUP = NCORES // B  # 4 cores per batch

F32 = mybir.dt.float32
BF16 = mybir.dt.bfloat16
AF = mybir.ActivationFunctionType
ALU = mybir.AluOpType

NEG_BIG = -1.0e30
LN_EPS = 1e-5

LAST_EXEC_NS = None

_CACHED = {}


def _build():
    nc = bacc.Bacc(None, target_bir_lowering=False)

    xqt = nc.dram_tensor("xqt", [D, RPC], BF16, kind="ExternalInput")
    xkt = nc.dram_tensor("xkt", [D, SK], BF16, kind="ExternalInput")
    xvt = nc.dram_tensor("xvt", [D, SK], BF16, kind="ExternalInput")
    wq = nc.dram_tensor("wq", [D, H * HS], BF16, kind="ExternalInput")
    wk = nc.dram_tensor("wk", [D, H * HS], BF16, kind="ExternalInput")
    wv = nc.dram_tensor("wv", [D, H * HS], BF16, kind="ExternalInput")
    wo = nc.dram_tensor("wo", [H * HS, ED], BF16, kind="ExternalInput")
    bq = nc.dram_tensor("bq", [128, NOT], F32, kind="ExternalInput")
    bk = nc.dram_tensor("bk", [128, NOT], F32, kind="ExternalInput")
    bv = nc.dram_tensor("bv", [1, H * HS], F32, kind="ExternalInput")
    pos = nc.dram_tensor("pos", [SK, H], F32, kind="ExternalInput")
    maskf = nc.dram_tensor("maskf", [SK], F32, kind="ExternalInput")
    lng = nc.dram_tensor("lng", [1, ED], F32, kind="ExternalInput")
    lnb = nc.dram_tensor("lnb", [1, ED], F32, kind="ExternalInput")
    out = nc.dram_tensor("out", [RPC, ED], F32, kind="ExternalOutput")

    with tile.TileContext(nc) as tc, ExitStack() as ctx:
        consts = ctx.enter_context(tc.tile_pool(name="consts", bufs=1))
        xpool = ctx.enter_context(tc.tile_pool(name="xpool", bufs=1))
        wsm = ctx.enter_context(tc.tile_pool(name="wsm", bufs=6))
        wbg = ctx.enter_context(tc.tile_pool(name="wbg", bufs=6))
        evac = ctx.enter_context(tc.tile_pool(name="evac", bufs=3))
        qzpool = ctx.enter_context(tc.tile_pool(name="qzpool", bufs=1))
        ppool = ctx.enter_context(tc.tile_pool(name="ppool", bufs=6))
        kvtiles = ctx.enter_context(tc.tile_pool(name="kvtiles", bufs=8))
        ypool = ctx.enter_context(tc.tile_pool(name="ypool", bufs=4))
        pss = ctx.enter_context(tc.tile_pool(name="pss", bufs=5, space="PSUM"))
        psz = ctx.enter_context(tc.tile_pool(name="psz", bufs=3, space="PSUM"))
        dram = ctx.enter_context(tc.tile_pool(name="dram", bufs=1, space="DRAM"))

        # ---- constants ----
        bq_sb = consts.tile([128, NOT], F32)
        nc.sync.dma_start(out=bq_sb, in_=bq[:, :])
        bk_sb = consts.tile([128, NOT], F32)
        nc.sync.dma_start(out=bk_sb, in_=bk[:, :])
        bv_bc = consts.tile([128, H * HS], F32)
        nc.sync.dma_start(out=bv_bc, in_=bv[:, :].to_broadcast([128, H * HS]))
        g_bc = consts.tile([128, ED], F32)
        nc.sync.dma_start(out=g_bc, in_=lng[:, :].to_broadcast([128, ED]))
        b_bc = consts.tile([128, ED], F32)
        nc.sync.dma_start(out=b_bc, in_=lnb[:, :].to_broadcast([128, ED]))
        pos_sb = consts.tile([128, NKT, H], F32)
        nc.sync.dma_start(out=pos_sb, in_=pos[:, :].rearrange("(kt p) h -> p kt h", p=128))
        mask_sb = consts.tile([128, NKT], F32)
        nc.sync.dma_start(out=mask_sb, in_=maskf[:].rearrange("(kt p) -> p kt", p=128))
        eps_sb = consts.tile([128, 1], F32)
        nc.vector.memset(eps_sb, LN_EPS)

        maskadd = consts.tile([128, NKT], F32)
        nc.vector.tensor_scalar(
            out=maskadd, in0=mask_sb, scalar1=1.0, scalar2=-NEG_BIG,
            op0=ALU.subtract, op1=ALU.mult,
        )
        bias_sb = consts.tile([128, NKT, H], F32)
        for h in range(H):
            nc.vector.tensor_add(
                out=bias_sb[:, :, h], in0=pos_sb[:, :, h], in1=maskadd[:, :]
            )

        # ---- resident activations (bf16) ----
        xk_res = xpool.tile([128, NDT, SK], BF16)
        nc.sync.dma_start(out=xk_res, in_=xkt[:, :].rearrange("(t p) r -> p t r", p=128))
        xv_res = xpool.tile([128, NDT, SK], BF16)
        nc.sync.dma_start(out=xv_res, in_=xvt[:, :].rearrange("(t p) r -> p t r", p=128))
        xqt_sb = xpool.tile([128, NDT, RPC], BF16)
        nc.sync.dma_start(out=xqt_sb, in_=xqt[:, :].rearrange("(t p) r -> p t r", p=128))

        qT_sb = qzpool.tile([128, NOT, RPC], BF16)  # q^T  [hs, rows]
        zT_sb = qzpool.tile([128, NOT, RPC], BF16)  # z^T  [hs, rows]

        # full-batch k^T / v(+ones) in local DRAM
        kT_b = dram.tile([H * HS, SK], BF16)
        v_b = dram.tile([SK, H * 65], BF16)

        # ---- K projection, full batch (transposed layout out) ----
        for t in range(NOT):
            psk = [pss.tile([128, 512], F32, tag="ps", name=f"psk{t}_{_i}")
                   for _i in range(NRB)]
            for dt in range(NDT):
                wt = wsm.tile([128, 128], BF16, tag="w", name=f"wkt{t}_{dt}")
                nc.sync.dma_start(
                    out=wt, in_=wk[dt * 128:(dt + 1) * 128, t * 128:(t + 1) * 128]
                )
                for rb in range(NRB):
                    nc.tensor.matmul(
                        psk[rb], lhsT=wt, rhs=xk_res[:, dt, 512 * rb:512 * (rb + 1)],
                        start=(dt == 0), stop=(dt == NDT - 1),
                    )
            for rb in range(NRB):
                kt_ev = evac.tile([128, 512], BF16, tag="ev", name=f"kev{t}_{rb}")
                nc.vector.tensor_scalar_add(
                    out=kt_ev, in0=psk[rb], scalar1=bk_sb[:, t:t + 1]
                )
                nc.gpsimd.dma_start(
                    out=kT_b[t * 128:(t + 1) * 128, 512 * rb:512 * (rb + 1)],
                    in_=kt_ev,
                )

        # ---- V projection, full batch (natural layout, 65-wide head blocks
        # with a ones column -> z matmul accumulates softmax denominators) ----
        for rb in range(NRB):
            for half in range(2):
                psv = [pss.tile([128, 512], F32, tag="ps", name=f"psv{rb}_{half}_{_i}")
                       for _i in range(4)]
                for dt in range(NDT):
                    wvt = wbg.tile([128, 512], BF16, tag="w", name=f"wvt{rb}_{half}_{dt}")
                    nc.sync.dma_start(
                        out=wvt,
                        in_=wv[dt * 128:(dt + 1) * 128, half * 512:(half + 1) * 512],
                    )
                    for rt in range(4):
                        nc.tensor.matmul(
                            psv[rt],
                            lhsT=xv_res[:, dt, 512 * rb + 128 * rt:512 * rb + 128 * (rt + 1)],
                            rhs=wvt,
                            start=(dt == 0), stop=(dt == NDT - 1),
                        )
                for rt in range(4):
                    v_ev = evac.tile([128, 8, 65], BF16, tag="ev", name=f"vev{rb}_{half}_{rt}")
                    nc.vector.tensor_add(
                        out=v_ev[:, :, 0:64],
                        in0=psv[rt][:, :].rearrange("p (h c) -> p h c", c=64),
                        in1=bv_bc[:, half * 512:(half + 1) * 512].rearrange(
                            "p (h c) -> p h c", c=64),
                    )
                    nc.vector.memset(v_ev[:, :, 64:65].bitcast(mybir.dt.uint16), 0x3F80)
                    nc.gpsimd.dma_start(
                        out=v_b[512 * rb + 128 * rt:512 * rb + 128 * (rt + 1),
                                half * 520:(half + 1) * 520],
                        in_=v_ev,
                    )

        # ---- Q projection (own rows, transposed layout out) ----
        for t in range(NOT):
            ps = pss.tile([128, RPC], F32, tag="ps", name=f"psq{t}")
            for dt in range(NDT):
                wt = wsm.tile([128, 128], BF16, tag="w", name=f"wqt{t}_{dt}")
                nc.sync.dma_start(
                    out=wt, in_=wq[dt * 128:(dt + 1) * 128, t * 128:(t + 1) * 128]
                )
                nc.tensor.matmul(
                    ps, lhsT=wt, rhs=xqt_sb[:, dt, :],
                    start=(dt == 0), stop=(dt == NDT - 1),
                )
            nc.vector.tensor_scalar_add(
                out=qT_sb[:, t, :], in0=ps, scalar1=bq_sb[:, t:t + 1]
            )

        # ---- attention per head-pair ----
        for j in range(H // 2):
            pzs = [psz.tile([65, RPC], F32, tag="pz", name=f"pz{j}_{_i}")
                   for _i in range(2)]
            for kt in range(NKT):
                kpair = kvtiles.tile([128, 128], BF16, tag="kt")
                nc.scalar.dma_start(
                    out=kpair,
                    in_=kT_b[128 * j:128 * (j + 1), 128 * kt:128 * (kt + 1)],
                )
                vpair = kvtiles.tile([128, 130], BF16, tag="vt")
                nc.gpsimd.dma_start(
                    out=vpair,
                    in_=v_b[128 * kt:128 * (kt + 1), 130 * j:130 * j + 130],
                )
                for hh in range(2):
                    h, sh = 2 * j + hh, 64 * hh
                    ps_s = pss.tile([128, RPC], F32, tag="ps", name=f"pss{j}_{kt}_{hh}")
                    nc.tensor.matmul(
                        ps_s,
                        lhsT=kpair[sh:sh + 64, :],
                        rhs=qT_sb[sh:sh + 64, j, :],
                        start=True, stop=True,
                    )
                    pt = ppool.tile([128, RPC], BF16, tag="p")
                    nc.scalar.activation(
                        out=pt, in_=ps_s, func=AF.Exp,
                        bias=bias_sb[:, kt, h:h + 1], scale=0.125,
                    )
                    nc.tensor.matmul(
                        pzs[hh],
                        lhsT=vpair[:, 65 * hh:65 * hh + 65],
                        rhs=pt,
                        start=(kt == 0), stop=(kt == NKT - 1),
                        skip_group_check=True,
                    )
            # normalize: d = pz[64] + 1 (dummy); z /= d
            for hh in range(2):
                d_sb = evac.tile([1, RPC], F32, tag="d", name=f"d{j}_{hh}", bufs=2)
                nc.vector.tensor_scalar_add(
                    out=d_sb, in0=pzs[hh][64:65, :], scalar1=1.0
                )
                r_sb = evac.tile([1, RPC], F32, tag="r", name=f"r{j}_{hh}", bufs=2)
                nc.vector.reciprocal(out=r_sb, in_=d_sb)
                rb_sb = evac.tile([64, RPC], F32, tag="rb", name=f"rb{j}_{hh}", bufs=2)
                nc.gpsimd.partition_broadcast(rb_sb, r_sb)
                if hh == 0:
                    nc.vector.tensor_mul(
                        out=zT_sb[0:64, j, :], in0=pzs[hh][0:64, :], in1=rb_sb
                    )
                else:
                    zn = evac.tile([64, RPC], BF16, tag="zn", name=f"zn{j}", bufs=2)
                    nc.vector.tensor_mul(
                        out=zn, in0=pzs[hh][0:64, :], in1=rb_sb
                    )
                    nc.sync.dma_start(out=zT_sb[64:128, j, :], in_=zn)

        # ---- out projection ----
        y_sb = [ypool.tile([128, ED], F32, tag="y", name=f"ysb{_i}") for _i in range(NRT)]
        for half in range(2):
            psy = [pss.tile([128, 512], F32, tag="ps", name=f"psy{half}_{_i}")
                   for _i in range(NRT)]
            for jj in range(NOT):
                wot = wbg.tile([128, 512], BF16, tag="w", name=f"wot{half}_{jj}")
                nc.sync.dma_start(
                    out=wot,
                    in_=wo[jj * 128:(jj + 1) * 128, half * 512:(half + 1) * 512],
                )
                for rt in range(NRT):
                    nc.tensor.matmul(
                        psy[rt],
                        lhsT=zT_sb[:, jj, rt * 128:(rt + 1) * 128],
                        rhs=wot,
                        start=(jj == 0), stop=(jj == NOT - 1),
                    )
            for rt in range(NRT):
                nc.vector.tensor_copy(
                    out=y_sb[rt][:, half * 512:(half + 1) * 512], in_=psy[rt]
                )

        # ---- LayerNorm + store ----
        for rt in range(NRT):
            y = y_sb[rt]
            stats = evac.tile([128, 2, 6], F32, tag="st", name=f"st{rt}", bufs=2)
            nc.vector.bn_stats(out=stats[:, 0, :], in_=y[:, 0:512])
            nc.vector.bn_stats(out=stats[:, 1, :], in_=y[:, 512:1024])
            mv = evac.tile([128, 2], F32, tag="mv", name=f"mv{rt}", bufs=2)
            nc.vector.bn_aggr(out=mv, in_=stats)
            std = evac.tile([128, 1], F32, tag="sd", name=f"sd{rt}", bufs=2)
            nc.scalar.activation(
                out=std, in_=mv[:, 1:2], func=AF.Sqrt, bias=eps_sb[:, 0:1]
            )
            rstd = evac.tile([128, 1], F32, tag="rs", name=f"rs{rt}", bufs=2)
            nc.vector.reciprocal(out=rstd, in_=std)
            nc.vector.tensor_scalar(
                out=y, in0=y, scalar1=mv[:, 0:1], scalar2=rstd,
                op0=ALU.subtract, op1=ALU.mult,
            )
            nc.vector.tensor_mul(out=y, in0=y, in1=g_bc)
            nc.vector.tensor_add(out=y, in0=y, in1=b_bc)
            nc.gpsimd.dma_start(out=out[rt * 128:(rt + 1) * 128, :], in_=y)

    return nc


def prep_in_maps(query, key, value, attention_mask, pos_attn_score,
                 W_Q, b_Q, W_K, b_K, W_V, b_V, W_O, ln_gamma, ln_beta):
    import ml_dtypes
    f32 = np.float32
    bf16 = ml_dtypes.bfloat16
    q2 = np.asarray(query, f32).reshape(B * SQ, D)
    k2 = np.asarray(key, f32).reshape(B * SK, D)
    v2 = np.asarray(value, f32).reshape(B * SK, D)
    wq2 = np.ascontiguousarray(np.asarray(W_Q, f32).transpose(2, 1, 0).reshape(D, H * HS)).astype(bf16)
    wk2 = np.ascontiguousarray(np.asarray(W_K, f32).transpose(2, 1, 0).reshape(D, H * HS)).astype(bf16)
    wv2 = np.ascontiguousarray(np.asarray(W_V, f32).transpose(2, 1, 0).reshape(D, H * HS)).astype(bf16)
    wo2 = np.ascontiguousarray(np.asarray(W_O, f32).transpose(1, 2, 0).reshape(H * HS, ED)).astype(bf16)
    bq2 = np.ascontiguousarray(np.asarray(b_Q, f32).reshape(NOT, 128).T)
    bk2 = np.ascontiguousarray(np.asarray(b_K, f32).reshape(NOT, 128).T)
    bv2 = np.ascontiguousarray(np.asarray(b_V, f32).reshape(1, H * HS))
    pos_np = np.asarray(pos_attn_score, f32)
    mask_np = np.asarray(attention_mask).astype(f32)
    lng = np.ascontiguousarray(np.asarray(ln_gamma, f32).reshape(1, ED))
    lnb = np.ascontiguousarray(np.asarray(ln_beta, f32).reshape(1, ED))

    kT_by_batch = [np.ascontiguousarray(k2[b * SK:(b + 1) * SK].T).astype(bf16)
                   for b in range(B)]
    vT_by_batch = [np.ascontiguousarray(v2[b * SK:(b + 1) * SK].T).astype(bf16)
                   for b in range(B)]

    in_maps = []
    for c in range(NCORES):
        b = c // GROUP
        rows = slice(RPC * c, RPC * (c + 1))
        in_maps.append({
            "xqt": np.ascontiguousarray(q2[rows].T).astype(bf16),
            "xkt": kT_by_batch[b],
            "xvt": vT_by_batch[b],
            "wq": wq2, "wk": wk2, "wv": wv2, "wo": wo2,
            "bq": bq2, "bk": bk2, "bv": bv2,
            "pos": np.ascontiguousarray(pos_np[b]),
            "maskf": np.ascontiguousarray(mask_np[b]),
            "lng": lng, "lnb": lnb,
        })
    return in_maps


def kernel(**inputs):
    global LAST_EXEC_NS
    in_maps = prep_in_maps(**inputs)
    if "nc" not in _CACHED:
        nc = _build()
        nc.finalize()
        _CACHED["nc"] = nc
    nc = _CACHED["nc"]

    trace = bool(os.environ.get("BASS_TRACE"))
    res = run_bass_kernel_spmd(nc, in_maps, core_ids=list(range(NCORES)),
                               trace=trace)
    LAST_EXEC_NS = res.exec_time_ns
    _CACHED["last_result"] = res

    out = np.empty((B * SQ, ED), np.float32)
    for c in range(NCORES):
        out[RPC * c:RPC * (c + 1)] = res.results[c]["out"]
    return out.reshape(B, SQ, ED)
